# revision 4
# baseline (speedup 1.0000x reference)
"""Windowed cross-attention (sparse_attention) on Trainium2.

Data-parallel over the batch axis across 8 NeuronCores. Each core processes
16 windows (4096 tokens). Host pre-transposes x/y to feature-major layout and
pre-gathers exp(rel_bias) per head so the device program is pure matmul +
softmax with no on-device transposes or gathers:

  qT = (q_w.T @ xT) * scale          (feature-major)
  kT = kv_w[:, :C].T @ yT            (feature-major)
  v  = yT.T-tiles @ kv_w[:, C:]      (token-major, ones column appended)
  attnT[k,q] = kT_h.T-slices @ qT_h  (PSUM)
  expT = exp(attnT) * exp(biasT)     (ACT + DVE)
  [outT_unnorm; denom] = [v|1].T @ expT    (one matmul, denom = row 64)
  outT = outT_unnorm * bcast(1/denom)      (rank-1 ones-matmul broadcast)
  finT = proj_w.T-slices @ outT + proj_b   (feature-major out, host transposes)
"""

import numpy as np

_TRN_REPO = "/opt/trn_rl_repo"
N_CORES = 8
B, NW, C = 128, 256, 512        # full batch, window tokens, channels
H, D = 8, 64                    # heads, head dim
WH = WW = 16
BC = B // N_CORES               # windows per core
T = BC * NW                     # tokens per core
NSB_FULL = 8                    # super-batches (2 windows each) per core
SBT = T // NSB_FULL             # tokens per super-batch


def build_module(reps=1, mm="float32r", nsb=NSB_FULL):
    """Build + compile the per-core Bass module (SPMD; same program all cores)."""
    import sys
    if _TRN_REPO not in sys.path:
        sys.path.insert(0, _TRN_REPO)
    from contextlib import ExitStack

    import concourse.bacc as bacc
    import concourse.tile as tile
    from concourse import mybir

    f32 = mybir.dt.float32
    mmdt = getattr(mybir.dt, mm)
    AF = mybir.ActivationFunctionType

    def mc(ap):  # matmul operands already carry the matmul dtype
        return ap

    nc = bacc.Bacc("TRN2", debug=False, enable_asserts=False, num_devices=N_CORES)
    xT_d = nc.dram_tensor("xT", [C, T], mmdt, kind="ExternalInput")
    yT_d = nc.dram_tensor("yT", [C, T], mmdt, kind="ExternalInput")
    qw_d = nc.dram_tensor("qw", [C, C], mmdt, kind="ExternalInput")
    kvw_d = nc.dram_tensor("kvw", [C, 2 * C], mmdt, kind="ExternalInput")
    pw_d = nc.dram_tensor("pw", [C, C], mmdt, kind="ExternalInput")
    pbT_d = nc.dram_tensor("pbT", [128, 4], f32, kind="ExternalInput")
    eb_d = nc.dram_tensor("eb", [H, NW, NW], f32, kind="ExternalInput")
    ones64_d = nc.dram_tensor("ones64_in", [1, 64], mmdt, kind="ExternalInput")
    onescol_d = nc.dram_tensor("onescol", [128, H, 1], mmdt, kind="ExternalInput")
    outT_d = nc.dram_tensor("outT", [C, T], f32, kind="ExternalOutput")

    xT, yT, outT = xT_d.ap(), yT_d.ap(), outT_d.ap()

    with tile.TileContext(nc) as tc, ExitStack() as ctx:
        ctx.enter_context(nc.allow_low_precision(
            reason="float32r is the matmul input format; accumulation stays fp32"))
        consts = ctx.enter_context(tc.tile_pool(name="consts", bufs=1))
        xy_pool = ctx.enter_context(tc.tile_pool(name="xy", bufs=2))
        qkv_pool = ctx.enter_context(tc.tile_pool(name="qkv", bufs=2))
        exp_pool = ctx.enter_context(tc.tile_pool(name="expp", bufs=8))
        oT_pool = ctx.enter_context(tc.tile_pool(name="oT", bufs=2))
        fin_pool = ctx.enter_context(tc.tile_pool(name="fin", bufs=4))
        small = ctx.enter_context(tc.tile_pool(name="small", bufs=8))
        pp = ctx.enter_context(tc.tile_pool(name="pp", bufs=3, space="PSUM"))
        attp = ctx.enter_context(tc.tile_pool(name="attp", bufs=2, space="PSUM"))
        op = ctx.enter_context(tc.tile_pool(name="op", bufs=2, space="PSUM"))
        bp = ctx.enter_context(tc.tile_pool(name="bp", bufs=1, space="PSUM"))

        # ---- constants: weights, bias table, ones vector ----
        qw_t, kvw_t, pw_t, eb_t = [], [], [], []
        for i in range(4):
            t = consts.tile([128, C], mmdt, name=f"qw{i}", tag=f"qw{i}")
            nc.sync.dma_start(t[:], qw_d.ap()[i * 128:(i + 1) * 128, :])
            qw_t.append(t)
        for i in range(4):
            t = consts.tile([128, 2 * C], mmdt, name=f"kvw{i}", tag=f"kvw{i}")
            nc.sync.dma_start(t[:], kvw_d.ap()[i * 128:(i + 1) * 128, :])
            kvw_t.append(t)
        for i in range(4):
            t = consts.tile([128, C], mmdt, name=f"pw{i}", tag=f"pw{i}")
            nc.sync.dma_start(t[:], pw_d.ap()[i * 128:(i + 1) * 128, :])
            pw_t.append(t)
        for h in range(H):
            for kt in range(2):
                t = consts.tile([128, NW], f32, name=f"eb{h}_{kt}", tag=f"eb{h}_{kt}")
                nc.sync.dma_start(t[:], eb_d.ap()[h, kt * 128:(kt + 1) * 128, :])
                eb_t.append(t)
        pbT_t = consts.tile([128, 4], f32, name="pbT", tag="pbT")
        nc.sync.dma_start(pbT_t[:], pbT_d.ap())
        ones64 = consts.tile([1, 64], mmdt, name="ones64", tag="ones64")
        nc.sync.dma_start(ones64[:], ones64_d.ap())

        def do_sb(sb):
            ts = sb * SBT
            # ---- load activations (feature-major) ----
            xt, yt = [], []
            for kin in range(4):
                t = xy_pool.tile([128, SBT], mmdt, name=f"xt_{sb}_{kin}",
                                 tag=f"xt{kin}")
                nc.sync.dma_start(t[:], xT[kin * 128:(kin + 1) * 128, ts:ts + SBT])
                xt.append(t)
            for kin in range(4):
                t = xy_pool.tile([128, SBT], mmdt, name=f"yt_{sb}_{kin}",
                                 tag=f"yt{kin}")
                nc.sync.dma_start(t[:], yT[kin * 128:(kin + 1) * 128, ts:ts + SBT])
                yt.append(t)

            # ---- q projection (feature-major, fold in softmax scale) ----
            qT = []
            for m in range(4):
                ps = pp.tile([128, SBT], f32, name=f"qps_{sb}_{m}", tag="pp")
                for kin in range(4):
                    nc.tensor.matmul(ps[:], mc(qw_t[kin][:, m * 128:(m + 1) * 128]),
                                     mc(xt[kin][:]), start=(kin == 0), stop=(kin == 3))
                qm = qkv_pool.tile([128, SBT], mmdt, name=f"qT_{sb}_{m}", tag=f"q{m}")
                nc.scalar.activation(qm[:], ps[:], AF.Copy, scale=float(D) ** -0.5)
                qT.append(qm)

            # ---- k projection (feature-major) ----
            kT = []
            for m in range(4):
                ps = pp.tile([128, SBT], f32, name=f"kps_{sb}_{m}", tag="pp")
                for kin in range(4):
                    nc.tensor.matmul(ps[:], mc(kvw_t[kin][:, m * 128:(m + 1) * 128]),
                                     mc(yt[kin][:]), start=(kin == 0), stop=(kin == 3))
                km = qkv_pool.tile([128, SBT], mmdt, name=f"kT_{sb}_{m}", tag=f"k{m}")
                nc.scalar.activation(km[:], ps[:], AF.Copy)
                kT.append(km)

            # ---- v projection (token-major) + ones column per head ----
            vo = []
            for mt in range(4):
                ps = pp.tile([128, C], f32, name=f"vps_{sb}_{mt}", tag="pp")
                for kin in range(4):
                    nc.tensor.matmul(ps[:], mc(yt[kin][:, mt * 128:(mt + 1) * 128]),
                                     mc(kvw_t[kin][:, C:2 * C]),
                                     start=(kin == 0), stop=(kin == 3))
                vt = qkv_pool.tile([128, H, D + 1], mmdt, name=f"vo_{sb}_{mt}",
                                   tag=f"vo{mt}")
                nc.sync.dma_start(vt[:, :, D:D + 1], onescol_d.ap())
                nc.vector.tensor_copy(vt[:, :, 0:D],
                                      ps[:].rearrange("p (h d) -> p h d", h=H))
                vo.append(vt)

            oT = []
            for m in range(4):
                t = oT_pool.tile([128, SBT], mmdt, name=f"oT_{sb}_{m}", tag=f"oT{m}")
                oT.append(t)

            # ---- attention: 2 windows x 8 heads ----
            for b2 in range(2):
                for h in range(H):
                    hp = (h % 2) * 64
                    ht = h // 2
                    ex = []
                    for kt in range(2):
                        aps = attp.tile([128, NW], f32,
                                        name=f"aps_{sb}_{b2}_{h}_{kt}", tag="attp")
                        lhsT = kT[ht][hp:hp + 64,
                                      b2 * NW + kt * 128:b2 * NW + (kt + 1) * 128]
                        rhs = qT[ht][hp:hp + 64, b2 * NW:(b2 + 1) * NW]
                        nc.tensor.matmul(aps[:], mc(lhsT), mc(rhs),
                                         start=True, stop=True)
                        e = exp_pool.tile([128, NW], mmdt,
                                          name=f"ex_{sb}_{b2}_{h}_{kt}", tag="ex")
                        nc.scalar.activation(e[:], aps[:], AF.Exp)
                        nc.vector.tensor_mul(e[:], e[:], eb_t[h * 2 + kt][:])
                        ex.append(e)
                    # unnormalized out + denominator in one PSUM tile
                    ops_t = op.tile([128, NW], f32, name=f"ops_{sb}_{b2}_{h}",
                                    tag="op")
                    for kt in range(2):
                        nc.tensor.matmul(ops_t[0:D + 1, :],
                                         mc(vo[b2 * 2 + kt][:, h, :]),
                                         mc(ex[kt][:]),
                                         start=(kt == 0), stop=(kt == 1))
                    r = small.tile([1, NW], mmdt, name=f"r_{sb}_{b2}_{h}", tag="r")
                    nc.vector.reciprocal(r[:], ops_t[D:D + 1, :])
                    bps = bp.tile([64, NW], f32, name=f"bps_{sb}_{b2}_{h}", tag="bp")
                    nc.tensor.matmul(bps[:], mc(ones64[:]), mc(r[:]),
                                     start=True, stop=True)
                    bc = exp_pool.tile([64, NW], f32, name=f"bc_{sb}_{b2}_{h}",
                                       tag="bc", bufs=4)
                    nc.scalar.activation(bc[:], bps[:], AF.Copy)
                    nc.vector.tensor_mul(
                        oT[ht][hp:hp + 64, b2 * NW:(b2 + 1) * NW],
                        ops_t[0:D, :], bc[:])

            # ---- output projection (feature-major) + bias ----
            for m in range(4):
                ps = pp.tile([128, SBT], f32, name=f"fps_{sb}_{m}", tag="pp")
                for kf in range(4):
                    nc.tensor.matmul(ps[:], mc(pw_t[kf][:, m * 128:(m + 1) * 128]),
                                     mc(oT[kf][:]), start=(kf == 0), stop=(kf == 3))
                fo = fin_pool.tile([128, SBT], f32, name=f"fo_{sb}_{m}", tag="fo")
                nc.vector.tensor_scalar_add(fo[:], ps[:], pbT_t[:, m:m + 1])
                nc.sync.dma_start(outT[m * 128:(m + 1) * 128, ts:ts + SBT], fo[:])

        def body():
            for sb in range(nsb):
                do_sb(sb)

        if reps == 1:
            body()
        else:
            with tc.For_i(0, reps, 1):
                body()

    nc.compile()
    return nc


def _rel_index():
    ch = np.arange(WH)
    cw = np.arange(WW)
    yy, xx = np.meshgrid(ch, cw, indexing="ij")
    coords = np.stack([yy, xx]).reshape(2, -1)           # [2, N]
    rel = coords[:, :, None] - coords[:, None, :]        # [2, N, N]
    idx = (rel[0] + WH - 1) * (2 * WW - 1) + (rel[1] + WW - 1)
    return idx                                           # [N, N] int


def make_in_maps(x, y, q_w, kv_w, proj_w, proj_b, bias_table):
    x = np.asarray(x, dtype=np.float32)
    y = np.asarray(y, dtype=np.float32)
    q_w = np.ascontiguousarray(np.asarray(q_w, dtype=np.float32))
    kv_w = np.ascontiguousarray(np.asarray(kv_w, dtype=np.float32))
    proj_w = np.ascontiguousarray(np.asarray(proj_w, dtype=np.float32))
    proj_b = np.asarray(proj_b, dtype=np.float32)
    bias_table = np.asarray(bias_table, dtype=np.float32)

    idx = _rel_index()
    rel_bias = bias_table[idx.reshape(-1)].reshape(NW, NW, H)   # [n1, n2, h]
    eb = np.ascontiguousarray(
        np.exp(rel_bias.transpose(2, 1, 0)), dtype=np.float32)  # [h, k=n2, q=n1]
    pbT = np.ascontiguousarray(proj_b.reshape(4, 128).T)        # [128, 4]

    in_maps = []
    for c in range(N_CORES):
        xc = x[c * BC:(c + 1) * BC].reshape(T, C)
        yc = y[c * BC:(c + 1) * BC].reshape(T, C)
        in_maps.append({
            "xT": np.ascontiguousarray(xc.T),
            "yT": np.ascontiguousarray(yc.T),
            "qw": q_w, "kvw": kv_w, "pw": proj_w, "pbT": pbT, "eb": eb,
            "ones64_in": np.ones((1, 64), np.float32),
            "onescol": np.ones((128, H, 1), np.float32),
        })
    return in_maps


_CACHE = {}


def kernel(x, y, q_w, kv_w, proj_w, proj_b, bias_table):
    import sys
    if _TRN_REPO not in sys.path:
        sys.path.insert(0, _TRN_REPO)
    from concourse.bass_utils import run_bass_kernel_spmd

    if "nc" not in _CACHE:
        _CACHE["nc"] = build_module()
    nc = _CACHE["nc"]

    in_maps = make_in_maps(x, y, q_w, kv_w, proj_w, proj_b, bias_table)
    res = run_bass_kernel_spmd(nc, in_maps, core_ids=list(range(N_CORES)))
    outs = [res.results[c]["outT"].T.reshape(BC, NW, C) for c in range(N_CORES)]
    return np.ascontiguousarray(np.concatenate(outs, axis=0), dtype=np.float32)
